# revision 8
# baseline (speedup 1.0000x reference)
"""SGCConv (dense linear + SpMM mean-aggregation) on 8 Trainium2 NeuronCores.

Strategy (data-parallel over the batch/graph dim, one graph per core):
  h = x @ W + b                     -> PE matmul (per 128-node tile, PE transpose for x^T)
  msgs = h[edge_col]                -> SWDGE dma_gather (exact, full DMA-bus rate)
  agg  = segment_sum(msgs, edge_row)-> edges are sorted by destination row on the host and
                                       packed into 128-edge chunks per 128-node window; each
                                       chunk is scattered with a one-hot matmul
                                       (onehot[k,m] = (rel_row[k]==m)) accumulated in PSUM.
                                       (dma_scatter_add is racy for duplicate indices on HW,
                                       so the scatter runs on the PE instead.)
  deg  = segment_sum(ones)          -> same one-hot, F=1 matmul into PSUM column 128
  out  = agg / clip(deg,1) * (node < num_nodes)  -> DVE, then DMA out.

edge_val is all-ones per the problem spec (fill: "ones"), so the per-edge value
multiply is folded away; degree is the plain edge count.
"""

import os
import sys

import numpy as np

for _p in ("/opt/trn_rl_repo", "/opt/pypackages"):
    if os.path.isdir(_p) and _p not in sys.path:
        sys.path.append(_p)

import concourse.bacc as bacc
import concourse.mybir as mybir
from concourse import bass_utils
from concourse.bass_interp import CoreSim
from concourse.tile import TileContext

B, N, E, C = 8, 10000, 320000, 128
TPW = 128                      # nodes per window
NW = (N + TPW - 1) // TPW      # 79 windows
NPAD = NW * TPW                # 10112

DT = mybir.dt.bfloat16         # matmul / gather dtype
DT_NP = np.dtype("bfloat16") if hasattr(np, "bfloat16") else None
if DT_NP is None:
    import ml_dtypes
    DT_NP = np.dtype(ml_dtypes.bfloat16)

_cache = {}
last_exec_time_ns = None
last_results = None


GI = 1024                      # idxs per dma_gather (SWDGE ring holds ~64 descs/engine)


def _build(kch: int, phase: str = ""):
    """Build + compile the single-core program (run SPMD on 8 cores)."""
    nchunks = NW * kch
    S = nchunks * 128          # padded edge slots
    SPAD = ((S + GI - 1) // GI) * GI
    ngath = SPAD // GI

    nc = bacc.Bacc("TRN2", target_bir_lowering=False, debug=False, num_devices=B,
                   num_swdge_queues=4)
    x_d = nc.dram_tensor("x", [N, C], mybir.dt.float32, kind="ExternalInput")
    w_d = nc.dram_tensor("w", [C, C], mybir.dt.float32, kind="ExternalInput")
    bias_d = nc.dram_tensor("bias", [1, C], mybir.dt.float32, kind="ExternalInput")
    colw_d = nc.dram_tensor("colw", [16, SPAD // 16], mybir.dt.int16, kind="ExternalInput")
    relw_d = nc.dram_tensor("relw", [128, nchunks], mybir.dt.float32, kind="ExternalInput")
    nn_d = nc.dram_tensor("nn", [1, 1], mybir.dt.float32, kind="ExternalInput")
    ident_d = nc.dram_tensor("ident", [128, 128], mybir.dt.float32, kind="ExternalInput")
    iotam_d = nc.dram_tensor("iotam", [128, 128], DT, kind="ExternalInput")
    iotap_d = nc.dram_tensor("iotap", [128, 1], mybir.dt.float32, kind="ExternalInput")
    out_d = nc.dram_tensor("out", [N, C], mybir.dt.float32, kind="ExternalOutput")
    h_d = nc.dram_tensor("h", [NPAD, C], DT, kind="Internal")

    eq = mybir.AluOpType.is_equal
    with TileContext(nc) as tc:
        with (
            tc.tile_pool(name="const", bufs=1) as pc,
            tc.tile_pool(name="work", bufs=3) as pw,
            tc.tile_pool(name="oh", bufs=8) as poh,
            tc.tile_pool(name="psA", bufs=2, space="PSUM") as psa,
            tc.tile_pool(name="psB", bufs=2, space="PSUM") as psb,
        ):
            # ---- constants ----
            w_sb32 = pc.tile([128, C], mybir.dt.float32)
            nc.sync.dma_start(w_sb32[:], w_d.ap())
            w_sb = pc.tile([128, C], DT)
            nc.vector.tensor_copy(w_sb[:], w_sb32[:])
            bias_row = pc.tile([1, C], mybir.dt.float32)
            nc.sync.dma_start(bias_row[:], bias_d.ap())
            bias_bc = pc.tile([128, C], mybir.dt.float32)
            nc.gpsimd.partition_broadcast(bias_bc[:], bias_row[:])
            ident = pc.tile([128, 128], mybir.dt.float32)
            nc.sync.dma_start(ident[:], ident_d.ap())
            iotam = pc.tile([128, 128], DT)
            nc.sync.dma_start(iotam[:], iotam_d.ap())
            iotap = pc.tile([128, 1], mybir.dt.float32)
            nc.sync.dma_start(iotap[:], iotap_d.ap())
            nn_row = pc.tile([1, 1], mybir.dt.float32)
            nc.sync.dma_start(nn_row[:], nn_d.ap())
            nn_bc = pc.tile([128, 1], mybir.dt.float32)
            nc.gpsimd.partition_broadcast(nn_bc[:], nn_row[:])
            ones_col = pc.tile([128, 1], DT)
            nc.gpsimd.memset(ones_col[:], 1.0)
            relw = pc.tile([128, nchunks], mybir.dt.float32)
            nc.sync.dma_start(relw[:], relw_d.ap())
            col_sb = pc.tile([128, SPAD // 16], mybir.dt.int16)
            nc.sync.dma_start(col_sb[0:16, :], colw_d.ap())
            nc.sync.dma_start(col_sb[16:32, :], col_sb[0:16, :])
            nc.sync.dma_start(col_sb[32:64, :], col_sb[0:32, :])
            nc.sync.dma_start(col_sb[64:128, :], col_sb[0:64, :])

            # ---- phase A: h = x @ W + b ----
            for t in range(NW):
                rw = min(128, N - TPW * t)
                x_t = pw.tile([128, C], mybir.dt.float32)
                if rw < 128:
                    nc.gpsimd.memset(x_t[:], 0.0)
                nc.sync.dma_start(x_t[0:rw, :], x_d.ap()[TPW * t : TPW * t + rw, :])
                ps_xt = psa.tile([128, 128], mybir.dt.float32, tag="ps_xt")
                nc.tensor.transpose(ps_xt[:], x_t[:], ident[:])
                xt_sb = pw.tile([128, 128], DT)
                nc.scalar.copy(xt_sb[:], ps_xt[:])
                ps_h = psa.tile([128, C], mybir.dt.float32, tag="ps_h")
                nc.tensor.matmul(ps_h[:], xt_sb[:], w_sb[:], start=True, stop=True)
                h_sb = pw.tile([128, C], DT)
                nc.vector.tensor_tensor(h_sb[:], ps_h[:], bias_bc[:], mybir.AluOpType.add)
                nc.sync.dma_start(h_d.ap()[TPW * t : TPW * (t + 1), :], h_sb[:])

            # ---- phase B: SpMM + normalize per 128-node window ----
            # Gathers are issued in GI-idx units (ring capacity) round-robin
            # over the 4 SWDGE queues; chunk j reads gather j//8 slice j%8.
            msgs_tiles = {}

            def get_msgs(g):
                if g not in msgs_tiles:
                    t = pw.tile([128, GI // 128, C], DT, tag="msgs", bufs=12)
                    nc.gpsimd.dma_gather(
                        t[:], h_d.ap(),
                        col_sb[:, g * (GI // 16) : (g + 1) * (GI // 16)],
                        GI, GI, C, queue_num=g % 4,
                    )
                    msgs_tiles[g] = t
                return msgs_tiles[g]

            for w in (range(NW) if phase != "A" else []):
                ps = psb.tile([128, C], mybir.dt.float32, tag="ps_agg")
                ps_deg = psb.tile([128, 1], mybir.dt.float32, tag="ps_deg")
                for k in range(kch):
                    j = w * kch + k
                    g, o = divmod(j, GI // 128)
                    msgs = get_msgs(g)
                    oh = poh.tile([128, 128], DT)
                    nc.vector.tensor_scalar(oh[:], iotam[:], relw[:, j : j + 1], None, eq)
                    nc.tensor.matmul(ps[:, 0:C], oh[:], msgs[:, o, :],
                                     start=(k == 0), stop=(k == kch - 1))
                    nc.tensor.matmul(ps_deg[:], oh[:], ones_col[:],
                                     start=(k == 0), stop=(k == kch - 1))
                # rscale = mask(node < nn) / clip(deg, 1)
                dclip = pw.tile([128, 1], mybir.dt.float32, tag="dclip")
                nc.vector.tensor_scalar(dclip[:], ps_deg[:], 1.0, None,
                                        mybir.AluOpType.max)
                rdeg = pw.tile([128, 1], mybir.dt.float32, tag="rdeg")
                nc.vector.reciprocal(rdeg[:], dclip[:])
                mask = pw.tile([128, 1], mybir.dt.float32, tag="mask")
                nc.vector.tensor_scalar(mask[:], iotap[:], float(TPW * w), nn_bc[:],
                                        mybir.AluOpType.add, mybir.AluOpType.is_lt)
                rsc = pw.tile([128, 1], mybir.dt.float32, tag="rsc")
                nc.vector.tensor_tensor(rsc[:], rdeg[:], mask[:], mybir.AluOpType.mult)
                o_sb = pw.tile([128, C], mybir.dt.float32, tag="o_sb")
                nc.vector.tensor_scalar(o_sb[:], ps[:, 0:C], rsc[:], None,
                                        mybir.AluOpType.mult)
                rw = min(128, N - TPW * w)
                nc.sync.dma_start(out_d.ap()[TPW * w : TPW * w + rw, :], o_sb[0:rw, :])

    nc.compile()
    return nc


def _prep_core(rows, cols, kch):
    """Sort edges by destination row, pack into per-window 128-edge chunks."""
    order = np.argsort(rows, kind="stable")
    rows_s = rows[order].astype(np.int64)
    cols_s = cols[order].astype(np.int64)
    win = rows_s // TPW
    cnt = np.bincount(win, minlength=NW)
    starts = np.zeros(NW + 1, np.int64)
    np.cumsum(cnt, out=starts[1:])
    rank = np.arange(rows_s.size, dtype=np.int64) - starts[win]
    slot = win * (kch * 128) + rank
    S = NW * kch * 128
    SPAD = ((S + 1023) // 1024) * 1024
    col_pad = np.zeros(SPAD, np.int16)
    col_pad[slot] = cols_s
    rel_pad = np.full(S, -1.0, np.float32)
    rel_pad[slot] = (rows_s - win * TPW).astype(np.float32)
    colw = np.ascontiguousarray(col_pad.reshape(SPAD // 16, 16).T)
    relw = np.ascontiguousarray(rel_pad.reshape(S // 128, 128).T)
    return colw, relw


def kernel(x, weight, bias, edge_row, edge_col, edge_val, num_nodes):
    global last_exec_time_ns, last_results
    x = np.asarray(x, np.float32)
    weight = np.asarray(weight, np.float32)
    bias = np.asarray(bias, np.float32)
    edge_row = np.asarray(edge_row)
    edge_col = np.asarray(edge_col)
    num_nodes = np.asarray(num_nodes)

    # max edges per 128-node window over all graphs -> uniform chunk count
    kch = 0
    for b in range(B):
        cnt = np.bincount(edge_row[b].astype(np.int64) // TPW, minlength=NW)
        kch = max(kch, int(cnt.max()))
    kch = (kch + 127) // 128

    phase = os.environ.get("SGC_PHASE", "")
    if (kch, phase) not in _cache:
        _cache[(kch, phase)] = _build(kch, phase)
    nc = _cache[(kch, phase)]

    ident = np.eye(128, dtype=np.float32)
    iotam = np.broadcast_to(np.arange(128, dtype=np.float32), (128, 128)).astype(DT_NP)
    iotap = np.arange(128, dtype=np.float32).reshape(128, 1)
    in_maps = []
    for b in range(B):
        colw, relw = _prep_core(edge_row[b], edge_col[b], kch)
        in_maps.append({
            "x": x[b], "w": weight, "bias": bias.reshape(1, C),
            "colw": colw, "relw": relw,
            "nn": np.array([[float(num_nodes[b])]], np.float32),
            "ident": ident, "iotam": np.ascontiguousarray(iotam), "iotap": iotap,
        })

    if os.environ.get("SGC_SIM"):
        sim = CoreSim(nc, trace=False)
        for k, v in in_maps[0].items():
            sim.tensor(k)[:] = v
        sim.tensor("out")[:] = 0
        sim.simulate()
        out0 = np.array(sim.tensor("out"))
        out = np.zeros((B, N, C), np.float32)
        out[0] = out0
        return out

    res = bass_utils.run_bass_kernel_spmd(
        nc, in_maps, core_ids=list(range(B)),
        trace=bool(os.environ.get("SGC_TRACE")),
        tmpdir=os.environ.get("SGC_TRACE_DIR"),
    )
    last_exec_time_ns = res.exec_time_ns
    last_results = res
    return np.stack([res.results[b]["out"] for b in range(B)])


# revision 9
# speedup vs baseline: 1.7771x; 1.7771x over previous
"""SGCConv (dense linear + SpMM mean-aggregation) on 8 Trainium2 NeuronCores.

Strategy (data-parallel over the batch/graph dim, one graph per core):
  h = x @ W + b                     -> PE matmul (per 128-node tile, PE transpose for x^T)
  msgs = h[edge_col]                -> SWDGE dma_gather (exact, full DMA-bus rate)
  agg  = segment_sum(msgs, edge_row)-> edges are sorted by destination row on the host and
                                       packed into 128-edge chunks per 128-node window; each
                                       chunk is scattered with a one-hot matmul
                                       (onehot[k,m] = (rel_row[k]==m)) accumulated in PSUM.
                                       (dma_scatter_add is racy for duplicate indices on HW,
                                       so the scatter runs on the PE instead.)
  deg  = segment_sum(ones)          -> same one-hot, F=1 matmul into PSUM column 128
  out  = agg / clip(deg,1) * (node < num_nodes)  -> DVE, then DMA out.

edge_val is all-ones per the problem spec (fill: "ones"), so the per-edge value
multiply is folded away; degree is the plain edge count.
"""

import os
import sys

import numpy as np

for _p in ("/opt/trn_rl_repo", "/opt/pypackages"):
    if os.path.isdir(_p) and _p not in sys.path:
        sys.path.append(_p)

import concourse.bacc as bacc
import concourse.mybir as mybir
from concourse import bass_utils
from concourse.bass_interp import CoreSim
from concourse.tile import TileContext

B, N, E, C = 8, 10000, 320000, 128
TPW = 128                      # nodes per window
NW = (N + TPW - 1) // TPW      # 79 windows
NPAD = NW * TPW                # 10112

DT = mybir.dt.bfloat16         # matmul / gather dtype
DT_NP = np.dtype("bfloat16") if hasattr(np, "bfloat16") else None
if DT_NP is None:
    import ml_dtypes
    DT_NP = np.dtype(ml_dtypes.bfloat16)

_cache = {}
last_exec_time_ns = None
last_results = None


GI = 1024                      # idxs per dma_gather (SWDGE ring holds ~64 descs/engine)


def _build(kch: int, phase: str = ""):
    """Build + compile the single-core program (run SPMD on 8 cores)."""
    nchunks = NW * kch
    S = nchunks * 128          # padded edge slots
    SPAD = ((S + GI - 1) // GI) * GI
    ngath = SPAD // GI

    nc = bacc.Bacc("TRN2", target_bir_lowering=False, debug=False, num_devices=B,
                   num_swdge_queues=4)
    x_d = nc.dram_tensor("x", [N, C], mybir.dt.float32, kind="ExternalInput")
    w_d = nc.dram_tensor("w", [C, C], mybir.dt.float32, kind="ExternalInput")
    bias_d = nc.dram_tensor("bias", [1, C], mybir.dt.float32, kind="ExternalInput")
    colw_d = nc.dram_tensor("colw", [16, SPAD // 16], mybir.dt.int16, kind="ExternalInput")
    relw_d = nc.dram_tensor("relw", [128, nchunks], DT, kind="ExternalInput")
    nn_d = nc.dram_tensor("nn", [1, 1], mybir.dt.float32, kind="ExternalInput")
    ident_d = nc.dram_tensor("ident", [128, 128], mybir.dt.float32, kind="ExternalInput")
    iotam_d = nc.dram_tensor("iotam", [128, kch * 128], DT, kind="ExternalInput")
    iotap_d = nc.dram_tensor("iotap", [128, 1], mybir.dt.float32, kind="ExternalInput")
    out_d = nc.dram_tensor("out", [N, C], mybir.dt.float32, kind="ExternalOutput")
    h_d = nc.dram_tensor("h", [NPAD, C], DT, kind="Internal")

    eq = mybir.AluOpType.is_equal
    with TileContext(nc) as tc:
        with (
            tc.tile_pool(name="const", bufs=1) as pc,
            tc.tile_pool(name="work", bufs=3) as pw,
            tc.tile_pool(name="oh", bufs=3) as poh,
            tc.tile_pool(name="psA", bufs=2, space="PSUM") as psa,
            tc.tile_pool(name="psB", bufs=2, space="PSUM") as psb,
        ):
            # ---- constants ----
            w_sb32 = pc.tile([128, C], mybir.dt.float32)
            nc.sync.dma_start(w_sb32[:], w_d.ap())
            w_sb = pc.tile([128, C], DT)
            nc.vector.tensor_copy(w_sb[:], w_sb32[:])
            bias_row = pc.tile([1, C], mybir.dt.float32)
            nc.sync.dma_start(bias_row[:], bias_d.ap())
            bias_bc = pc.tile([128, C], mybir.dt.float32)
            nc.gpsimd.partition_broadcast(bias_bc[:], bias_row[:])
            ident = pc.tile([128, 128], mybir.dt.float32)
            nc.sync.dma_start(ident[:], ident_d.ap())
            iotam = pc.tile([128, kch * 128], DT)
            nc.sync.dma_start(iotam[:], iotam_d.ap())
            iotap = pc.tile([128, 1], mybir.dt.float32)
            nc.sync.dma_start(iotap[:], iotap_d.ap())
            nn_row = pc.tile([1, 1], mybir.dt.float32)
            nc.sync.dma_start(nn_row[:], nn_d.ap())
            nn_bc = pc.tile([128, 1], mybir.dt.float32)
            nc.gpsimd.partition_broadcast(nn_bc[:], nn_row[:])
            ones_col = pc.tile([128, 1], DT)
            nc.gpsimd.memset(ones_col[:], 1.0)
            relw = pc.tile([128, nchunks], DT)
            nc.sync.dma_start(relw[:], relw_d.ap())
            col_sb = pc.tile([128, SPAD // 16], mybir.dt.int16)
            nc.sync.dma_start(col_sb[0:16, :], colw_d.ap())
            nc.sync.dma_start(col_sb[16:32, :], col_sb[0:16, :])
            nc.sync.dma_start(col_sb[32:64, :], col_sb[0:32, :])
            nc.sync.dma_start(col_sb[64:128, :], col_sb[0:64, :])

            # ---- phase A: h = x @ W + b ----
            for t in range(NW):
                rw = min(128, N - TPW * t)
                x_t = pw.tile([128, C], mybir.dt.float32)
                if rw < 128:
                    nc.gpsimd.memset(x_t[:], 0.0)
                nc.sync.dma_start(x_t[0:rw, :], x_d.ap()[TPW * t : TPW * t + rw, :])
                ps_xt = psa.tile([128, 128], mybir.dt.float32, tag="ps_xt")
                nc.tensor.transpose(ps_xt[:], x_t[:], ident[:])
                xt_sb = pw.tile([128, 128], DT)
                nc.scalar.copy(xt_sb[:], ps_xt[:])
                ps_h = psa.tile([128, C], mybir.dt.float32, tag="ps_h")
                nc.tensor.matmul(ps_h[:], xt_sb[:], w_sb[:], start=True, stop=True)
                h_sb = pw.tile([128, C], DT)
                nc.vector.tensor_tensor(h_sb[:], ps_h[:], bias_bc[:], mybir.AluOpType.add)
                nc.sync.dma_start(h_d.ap()[TPW * t : TPW * (t + 1), :], h_sb[:])

            # ---- phase B: SpMM + normalize per 128-node window ----
            # Gathers are issued in GI-idx units (ring capacity) round-robin
            # over the 4 SWDGE queues; chunk j reads gather j//8 slice j%8.
            msgs_tiles = {}

            def get_msgs(g):
                if g not in msgs_tiles:
                    t = pw.tile([128, GI // 128, C], DT, tag="msgs", bufs=12)
                    nc.gpsimd.dma_gather(
                        t[:], h_d.ap(),
                        col_sb[:, g * (GI // 16) : (g + 1) * (GI // 16)],
                        GI, GI, C, queue_num=g % 4,
                    )
                    msgs_tiles[g] = t
                return msgs_tiles[g]

            for w in (range(NW) if phase != "A" else []):
                ps = psb.tile([128, C], mybir.dt.float32, tag="ps_agg")
                ps_deg = psb.tile([128, 1], mybir.dt.float32, tag="ps_deg")
                # all kch one-hots of the window in one DVE op:
                # oh[p, k, m] = (rel[p, w*kch+k] == m)
                oh = poh.tile([128, kch, 128], DT)
                nc.vector.tensor_tensor(
                    oh[:],
                    iotam[:].rearrange("p (k m) -> p k m", m=128),
                    relw[:, w * kch : (w + 1) * kch].to_broadcast([128, kch, 128]),
                    eq,
                )
                for k in range(kch):
                    j = w * kch + k
                    g, o = divmod(j, GI // 128)
                    msgs = get_msgs(g)
                    nc.tensor.matmul(ps[:, 0:C], oh[:, k, :], msgs[:, o, :],
                                     start=(k == 0), stop=(k == kch - 1))
                    nc.tensor.matmul(ps_deg[:], oh[:, k, :], ones_col[:],
                                     start=(k == 0), stop=(k == kch - 1))
                # rscale = mask(node < nn) / clip(deg, 1)
                dclip = pw.tile([128, 1], mybir.dt.float32, tag="dclip")
                nc.vector.tensor_scalar(dclip[:], ps_deg[:], 1.0, None,
                                        mybir.AluOpType.max)
                rdeg = pw.tile([128, 1], mybir.dt.float32, tag="rdeg")
                nc.vector.reciprocal(rdeg[:], dclip[:])
                mask = pw.tile([128, 1], mybir.dt.float32, tag="mask")
                nc.vector.tensor_scalar(mask[:], iotap[:], float(TPW * w), nn_bc[:],
                                        mybir.AluOpType.add, mybir.AluOpType.is_lt)
                rsc = pw.tile([128, 1], mybir.dt.float32, tag="rsc")
                nc.vector.tensor_tensor(rsc[:], rdeg[:], mask[:], mybir.AluOpType.mult)
                o_sb = pw.tile([128, C], mybir.dt.float32, tag="o_sb")
                nc.vector.tensor_scalar(o_sb[:], ps[:, 0:C], rsc[:], None,
                                        mybir.AluOpType.mult)
                rw = min(128, N - TPW * w)
                nc.sync.dma_start(out_d.ap()[TPW * w : TPW * w + rw, :], o_sb[0:rw, :])

    nc.compile()
    return nc


def _prep_core(rows, cols, kch):
    """Sort edges by destination row, pack into per-window 128-edge chunks."""
    order = np.argsort(rows, kind="stable")
    rows_s = rows[order].astype(np.int64)
    cols_s = cols[order].astype(np.int64)
    win = rows_s // TPW
    cnt = np.bincount(win, minlength=NW)
    starts = np.zeros(NW + 1, np.int64)
    np.cumsum(cnt, out=starts[1:])
    rank = np.arange(rows_s.size, dtype=np.int64) - starts[win]
    slot = win * (kch * 128) + rank
    S = NW * kch * 128
    SPAD = ((S + 1023) // 1024) * 1024
    col_pad = np.zeros(SPAD, np.int16)
    col_pad[slot] = cols_s
    rel_pad = np.full(S, -1.0, np.float32)
    rel_pad[slot] = (rows_s - win * TPW).astype(np.float32)
    colw = np.ascontiguousarray(col_pad.reshape(SPAD // 16, 16).T)
    relw = np.ascontiguousarray(rel_pad.reshape(S // 128, 128).T).astype(DT_NP)
    return colw, relw


def kernel(x, weight, bias, edge_row, edge_col, edge_val, num_nodes):
    global last_exec_time_ns, last_results
    x = np.asarray(x, np.float32)
    weight = np.asarray(weight, np.float32)
    bias = np.asarray(bias, np.float32)
    edge_row = np.asarray(edge_row)
    edge_col = np.asarray(edge_col)
    num_nodes = np.asarray(num_nodes)

    # max edges per 128-node window over all graphs -> uniform chunk count
    kch = 0
    for b in range(B):
        cnt = np.bincount(edge_row[b].astype(np.int64) // TPW, minlength=NW)
        kch = max(kch, int(cnt.max()))
    kch = (kch + 127) // 128

    phase = os.environ.get("SGC_PHASE", "")
    if (kch, phase) not in _cache:
        _cache[(kch, phase)] = _build(kch, phase)
    nc = _cache[(kch, phase)]

    ident = np.eye(128, dtype=np.float32)
    iotam = np.broadcast_to(np.tile(np.arange(128, dtype=np.float32), kch),
                            (128, kch * 128)).astype(DT_NP)
    iotap = np.arange(128, dtype=np.float32).reshape(128, 1)
    in_maps = []
    for b in range(B):
        colw, relw = _prep_core(edge_row[b], edge_col[b], kch)
        in_maps.append({
            "x": x[b], "w": weight, "bias": bias.reshape(1, C),
            "colw": colw, "relw": relw,
            "nn": np.array([[float(num_nodes[b])]], np.float32),
            "ident": ident, "iotam": np.ascontiguousarray(iotam), "iotap": iotap,
        })

    if os.environ.get("SGC_SIM"):
        sim = CoreSim(nc, trace=False)
        for k, v in in_maps[0].items():
            sim.tensor(k)[:] = v
        sim.tensor("out")[:] = 0
        sim.simulate()
        out0 = np.array(sim.tensor("out"))
        out = np.zeros((B, N, C), np.float32)
        out[0] = out0
        return out

    res = bass_utils.run_bass_kernel_spmd(
        nc, in_maps, core_ids=list(range(B)),
        trace=bool(os.environ.get("SGC_TRACE")),
        tmpdir=os.environ.get("SGC_TRACE_DIR"),
    )
    last_exec_time_ns = res.exec_time_ns
    last_results = res
    return np.stack([res.results[b]["out"] for b in range(B)])
